# revision 10
# baseline (speedup 1.0000x reference)
"""Laplacian normalization kernel for Trainium2 (8 NeuronCores, SPMD).

out = D^-1/2 A D^-1/2 where D = diag(row sums of A), A: [8192, 8192] fp32.

Sharding: rows split across 8 cores (1024 rows each, 8 stripes of 128).

Single-load design (64MB HBM traffic/core vs 80MB for load-twice):
  pass 1: stream each stripe once as f32 chunks on alternating HWDGE
    rings. One scalar-engine ACT op per chunk does BOTH jobs: casts the
    chunk into a resident bf16 SBUF tile (16MB total — fits) and emits
    the per-partition row sum via accum_out. Loads are dispatched SIX
    chunks ahead of their casts in the scalar queue: a cast waiting on
    its DMA otherwise head-of-line-blocks the next load dispatch behind
    it (measured: collapses load BW from ~330 to ~160GB/s with the
    naive load/cast/load/cast interleave).
  TWO AllGathers of the isq vector halves (stripes 0-3 after ~half the
    loads, stripes 4-7 at the end) so the first AG's latency hides under
    the remaining loads and the second's under the first stores.
  pass 2: out = (res_bf16 * r[:,None]) * c[None,:] on the vector engine
    into f32 staging tiles, stored on alternating rings. No HBM re-read.

isq chunk writes ride the sync HWDGE ring, batched after the next
stripe's loads are already enqueued, so their sqrt-chain latency never
stalls a load sitting behind them in ring FIFO order. The gpsimd queue
holds ONLY [AG1, cb0, AG2, cb1] so a collective trigger can never queue
behind unrelated DMAs (the failure mode of the previous version).

bf16 residents cost ~0.2% relative error on A; tolerance is 2e-2.
"""

import sys

sys.path.insert(0, "/opt/trn_rl_repo")

import numpy as np

import concourse.bacc as bacc
import concourse.tile as tile
from concourse import mybir
from concourse.bass_utils import run_bass_kernel_spmd

N = 8192          # full matrix dim
CORES = 8
R = N // CORES    # rows per core: 1024
P = 128           # partitions
S = R // P        # row stripes per core: 8
QW = 2048         # load chunk width
NQ = N // QW      # chunks per stripe: 4
HW = 4096         # pass-2 column block width
HAG = R // 2      # isq elements per collective half: 512
F32 = mybir.dt.float32
BF16 = mybir.dt.bfloat16
MUL = mybir.AluOpType.mult
X = mybir.AxisListType.X
COPY = mybir.ActivationFunctionType.Copy

_CACHE = {}


def build_nc():
    if "nc" in _CACHE:
        return _CACHE["nc"]
    nc = bacc.Bacc(
        "TRN2", target_bir_lowering=False, debug=False, num_devices=CORES
    )
    a = nc.dram_tensor("a_block", [R, N], F32, kind="ExternalInput").ap()
    out = nc.dram_tensor("out_block", [R, N], F32, kind="ExternalOutput").ap()

    with tile.TileContext(nc) as tc:
        with (
            tc.tile_pool(name="dram", bufs=1, space="DRAM") as dram,
            tc.tile_pool(name="res", bufs=1) as res,
            tc.tile_pool(name="io", bufs=6) as io,
            tc.tile_pool(name="cpool", bufs=1) as cpool,
            tc.tile_pool(name="small", bufs=1) as small,
        ):
            isq_loc = [
                dram.tile([HAG], F32, name=f"isq_loc{g}") for g in range(2)
            ]
            isq_ag = [
                dram.tile(
                    [CORES * HAG], F32, addr_space="Shared", name=f"isq_ag{g}"
                )
                for g in range(2)
            ]

            part = small.tile([P, S * NQ], F32)   # per-chunk row sums
            deg = small.tile([P, S], F32)         # combined per-stripe sums
            isq_sb = small.tile([P, S], F32)      # 1/sqrt(deg)

            res_t = [
                res.tile([P, N], BF16, tag=f"res{s}", bufs=1, name=f"res{s}")
                for s in range(S)
            ]

            def finish_stripe(s):
                """partials -> deg -> 1/deg (vector, tiny, inline)."""
                nc.vector.reduce_sum(
                    out=deg[:, s : s + 1],
                    in_=part[:, s * NQ : s * NQ + NQ],
                    axis=X,
                )
                nc.vector.reciprocal(deg[:, s : s + 1], deg[:, s : s + 1])

            def sqrt_and_write_isq(g):
                """sqrt + isq chunk writes for stripes 4g..4g+3, batched at
                a point where their recip deps are long done, so neither
                the sqrts (scalar queue) nor the tiny writes (sync ring)
                ever stall a load dispatch sitting behind them."""
                for i in range(4):
                    s = 4 * g + i
                    nc.scalar.sqrt(isq_sb[:, s : s + 1], deg[:, s : s + 1])
                for i in range(4):
                    s = 4 * g + i
                    nc.sync.dma_start(
                        isq_loc[g][i * P : (i + 1) * P].unsqueeze(1),
                        isq_sb[:, s : s + 1],
                    )

            # ---- pass 1: load once, cast+reduce per chunk ----
            NU = S * NQ          # 32 chunk units
            LOOK = 6             # dispatch-ahead depth = io pool size
            tiles = [None] * NU

            def dispatch(u):
                s, q = divmod(u, NQ)
                t = io.tile([P, QW], F32, tag="io", name=f"io{u % LOOK}")
                tiles[u] = t
                ld = nc.sync if u % 2 == 0 else nc.scalar
                ld.dma_start(
                    t[:], a[s * P : (s + 1) * P, q * QW : (q + 1) * QW]
                )

            for u in range(LOOK):
                dispatch(u)
            for u in range(NU):
                if u + LOOK < NU:
                    dispatch(u + LOOK)
                if u == 16:
                    # early enough that the trigger's sem-lane peers
                    # (loads <= u+LOOK) all complete by ~80us
                    sqrt_and_write_isq(0)
                s, q = divmod(u, NQ)
                dst = res_t[s][:, q * QW : (q + 1) * QW]
                if u % 2 == 0:
                    # sync-ring chunks: ACT does cast + row-sum in one op
                    nc.scalar.activation(
                        dst, tiles[u][:], COPY,
                        accum_out=part[:, u : u + 1],
                    )
                else:
                    # scalar-ring chunks: cast + reduce on the (otherwise
                    # idle) vector engine, keeping ACT cadence slack
                    nc.vector.tensor_scalar(
                        out=dst, in0=tiles[u][:],
                        scalar1=1.0, scalar2=None, op0=MUL,
                    )
                    nc.vector.reduce_sum(
                        out=part[:, u : u + 1], in_=dst, axis=X
                    )
                if q == NQ - 1:
                    finish_stripe(s)
            sqrt_and_write_isq(1)

            ag_args = dict(replica_groups=[list(range(CORES))])

            # cb[g][h]: AG half g's column scales for output columns
            # [h*4096, (h+1)*4096), packed [P, 4*512] (bf16), replicated
            # across partitions.
            cb = [
                [
                    cpool.tile(
                        [P, HW // 2], BF16, tag=f"cb{g}{h}", bufs=1,
                        name=f"cb{g}{h}",
                    )
                    for h in range(N // HW)
                ]
                for g in range(2)
            ]

            def bcast_cb(g):
                for h in range(N // HW):
                    src = (
                        isq_ag[g][h * (HW // 2) : (h + 1) * (HW // 2)]
                        .rearrange("(m c) -> m c", c=HAG)
                        .unsqueeze(0)
                        .to_broadcast([P, HW // 1024, HAG])
                    )
                    nc.gpsimd.dma_start(
                        cb[g][h][:].rearrange("p (m c) -> p m c", c=HAG), src
                    )

            # gpsimd queue: [AG1, cb0, AG2, cb1] and nothing else
            nc.gpsimd.collective_compute(
                "AllGather", mybir.AluOpType.bypass,
                ins=[isq_loc[0][:].opt()], outs=[isq_ag[0][:].opt()],
                **ag_args,
            )
            bcast_cb(0)
            nc.gpsimd.collective_compute(
                "AllGather", mybir.AluOpType.bypass,
                ins=[isq_loc[1][:].opt()], outs=[isq_ag[1][:].opt()],
                **ag_args,
            )
            bcast_cb(1)

            # ---- pass 2: out = (res * r) * c, AG1-covered columns first ----
            def c3(ap, h, g):
                """AG-half-g columns of ap's 4096-col block h as
                [P, 4, 512]: within each 1024-col block, cols
                [g*512, (g+1)*512)."""
                return ap[:, h * HW : (h + 1) * HW].rearrange(
                    "p (m c) -> p m c", c=1024
                )[:, :, g * HAG : (g + 1) * HAG]

            nunit = 0
            for g in range(2):
                for s in range(S):
                    for h in range(N // HW):
                        st = io.tile([P, QW], F32, tag="io")
                        stv = st[:].rearrange("p (m c) -> p m c", c=HAG)
                        nc.vector.scalar_tensor_tensor(
                            out=stv,
                            in0=c3(res_t[s], h, g),
                            scalar=isq_sb[:, s : s + 1],
                            in1=cb[g][h][:].rearrange(
                                "p (m c) -> p m c", c=HAG
                            ),
                            op0=MUL,
                            op1=MUL,
                        )
                        std = nc.sync if nunit % 2 == 0 else nc.scalar
                        std.dma_start(
                            c3(out[s * P : (s + 1) * P, :], h, g), stv
                        )
                        nunit += 1

    nc.compile()
    _CACHE["nc"] = nc
    return nc


def kernel(adjacency_matrix):
    A = np.ascontiguousarray(np.asarray(adjacency_matrix, dtype=np.float32))
    assert A.shape == (N, N)
    nc = build_nc()
    in_maps = [
        {"a_block": np.ascontiguousarray(A[k * R : (k + 1) * R])}
        for k in range(CORES)
    ]
    res = run_bass_kernel_spmd(nc, in_maps, list(range(CORES)))
    return np.concatenate(
        [res.results[k]["out_block"] for k in range(CORES)], axis=0
    )


# revision 15
# speedup vs baseline: 1.0560x; 1.0560x over previous
"""Laplacian normalization kernel for Trainium2 (8 NeuronCores, SPMD).

out = D^-1/2 A D^-1/2 where D = diag(row sums of A), A: [8192, 8192] fp32.

Sharding: rows split across 8 cores (1024 rows each, 8 stripes of 128).

Single-load design (64MB HBM traffic/core vs 80MB for load-twice):
  pass 1: stream each stripe once as f32 chunks on alternating HWDGE
    rings. One scalar-engine ACT op per chunk does BOTH jobs: casts the
    chunk into a resident bf16 SBUF tile (16MB total — fits) and emits
    the per-partition row sum via accum_out. Loads are dispatched SIX
    chunks ahead of their casts in the scalar queue: a cast waiting on
    its DMA otherwise head-of-line-blocks the next load dispatch behind
    it (measured: collapses load BW from ~330 to ~160GB/s with the
    naive load/cast/load/cast interleave).
  TWO AllGathers of the isq vector halves (stripes 0-3 after ~half the
    loads, stripes 4-7 at the end) so the first AG's latency hides under
    the remaining loads and the second's under the first stores.
  pass 2: out = (res_bf16 * r[:,None]) * c[None,:] on the vector engine
    into f32 staging tiles, stored on alternating rings. No HBM re-read.

isq chunk writes ride the sync HWDGE ring, batched after the next
stripe's loads are already enqueued, so their sqrt-chain latency never
stalls a load sitting behind them in ring FIFO order. The gpsimd queue
holds ONLY [AG1, cb0, AG2, cb1] so a collective trigger can never queue
behind unrelated DMAs (the failure mode of the previous version).

bf16 residents cost ~0.2% relative error on A; tolerance is 2e-2.
"""

import sys

sys.path.insert(0, "/opt/trn_rl_repo")

import numpy as np

import concourse.bacc as bacc
import concourse.tile as tile
from concourse import mybir
from concourse.bass_utils import run_bass_kernel_spmd

N = 8192          # full matrix dim
CORES = 8
R = N // CORES    # rows per core: 1024
P = 128           # partitions
S = R // P        # row stripes per core: 8
QW = 2048         # load chunk width
NQ = N // QW      # chunks per stripe: 4
HW = 4096         # pass-2 column block width
HAG = R // 2      # isq elements per collective half: 512
F32 = mybir.dt.float32
BF16 = mybir.dt.bfloat16
MUL = mybir.AluOpType.mult
X = mybir.AxisListType.X
COPY = mybir.ActivationFunctionType.Copy

_CACHE = {}


def build_nc():
    if "nc" in _CACHE:
        return _CACHE["nc"]
    nc = bacc.Bacc(
        "TRN2", target_bir_lowering=False, debug=False, num_devices=CORES
    )
    a = nc.dram_tensor("a_block", [R, N], F32, kind="ExternalInput").ap()
    out = nc.dram_tensor("out_block", [R, N], F32, kind="ExternalOutput").ap()

    with tile.TileContext(nc) as tc:
        with (
            tc.tile_pool(name="dram", bufs=1, space="DRAM") as dram,
            tc.tile_pool(name="res", bufs=1) as res,
            tc.tile_pool(name="io", bufs=6) as io,
            tc.tile_pool(name="cpool", bufs=1) as cpool,
            tc.tile_pool(name="small", bufs=1) as small,
        ):
            isq_loc = [
                dram.tile([HAG], F32, name=f"isq_loc{g}") for g in range(2)
            ]
            isq_ag = [
                dram.tile(
                    [CORES * HAG], F32, addr_space="Shared", name=f"isq_ag{g}"
                )
                for g in range(2)
            ]

            part = small.tile([P, S * NQ], F32)   # per-chunk row sums
            deg = small.tile([P, S], F32)         # combined per-stripe sums
            isq_sb = small.tile([P, S], F32)      # 1/sqrt(deg)

            res_t = [
                res.tile([P, N], BF16, tag=f"res{s}", bufs=1, name=f"res{s}")
                for s in range(S)
            ]

            def finish_stripe(s):
                """partials -> deg -> 1/deg (vector, tiny, inline)."""
                nc.vector.reduce_sum(
                    out=deg[:, s : s + 1],
                    in_=part[:, s * NQ : s * NQ + NQ],
                    axis=X,
                )
                nc.vector.reciprocal(deg[:, s : s + 1], deg[:, s : s + 1])

            def sqrt_and_write_isq(g):
                """sqrt (scalar, deps long done -> no queue stall) + isq
                chunk writes on the SWDGE queue. SWDGE keeps the writes
                out of the load/store ring FIFOs AND gives their
                completion sems a private lane history, so the AllGather
                trigger's sem-lane thresholds can never alias with bulk
                loads (measured +30us trigger delay when they ride the
                sync ring)."""
                for i in range(4):
                    s = 4 * g + i
                    nc.scalar.sqrt(isq_sb[:, s : s + 1], deg[:, s : s + 1])
                for i in range(4):
                    s = 4 * g + i
                    nc.gpsimd.dma_start(
                        isq_loc[g][i * P : (i + 1) * P].unsqueeze(1),
                        isq_sb[:, s : s + 1],
                    )

            ag_args = dict(replica_groups=[list(range(CORES))])

            def emit_ag(g):
                nc.gpsimd.collective_compute(
                    "AllGather", mybir.AluOpType.bypass,
                    ins=[isq_loc[g][:].opt()], outs=[isq_ag[g][:].opt()],
                    **ag_args,
                )

            # ---- pass 1: load once, cast+reduce per chunk ----
            NU = S * NQ          # 32 chunk units
            LOOK = 6             # dispatch-ahead depth = io pool size
            tiles = [None] * NU

            def dispatch(u):
                s, q = divmod(u, NQ)
                t = io.tile([P, QW], F32, tag="io", name=f"io{u % LOOK}")
                tiles[u] = t
                ld = nc.sync if u % 2 == 0 else nc.scalar
                ld.dma_start(
                    t[:], a[s * P : (s + 1) * P, q * QW : (q + 1) * QW]
                )

            for u in range(LOOK):
                dispatch(u)
            for u in range(NU):
                if u + LOOK < NU:
                    dispatch(u + LOOK)
                if u == 16:
                    # stripes 0-3 done: ship their isq + AG1 immediately
                    sqrt_and_write_isq(0)
                    emit_ag(0)
                s, q = divmod(u, NQ)
                nc.scalar.activation(
                    res_t[s][:, q * QW : (q + 1) * QW],
                    tiles[u][:],
                    COPY,
                    accum_out=part[:, u : u + 1],
                )
                if q == NQ - 1:
                    finish_stripe(s)
            sqrt_and_write_isq(1)
            emit_ag(1)

            # cb[g][h]: AG half g's column scales for output columns
            # [h*4096, (h+1)*4096), packed [P, 4*512] (bf16), replicated
            # across partitions.
            cb = [
                [
                    cpool.tile(
                        [P, HW // 2], BF16, tag=f"cb{g}{h}", bufs=1,
                        name=f"cb{g}{h}",
                    )
                    for h in range(N // HW)
                ]
                for g in range(2)
            ]

            def bcast_cb(g):
                for h in range(N // HW):
                    src = (
                        isq_ag[g][h * (HW // 2) : (h + 1) * (HW // 2)]
                        .rearrange("(m c) -> m c", c=HAG)
                        .unsqueeze(0)
                        .to_broadcast([P, HW // 1024, HAG])
                    )
                    nc.gpsimd.dma_start(
                        cb[g][h][:].rearrange("p (m c) -> p m c", c=HAG), src
                    )

            # gpsimd queue so far: [isqw_g0, AG1, isqw_g1, AG2]; the cb
            # broadcasts go last so neither AG trigger can queue behind a
            # DMA that waits on a collective completion
            bcast_cb(0)
            bcast_cb(1)

            # ---- pass 2: out = (res * r) * c, AG1-covered columns first ----
            def c3(ap, h, g):
                """AG-half-g columns of ap's 4096-col block h as
                [P, 4, 512]: within each 1024-col block, cols
                [g*512, (g+1)*512)."""
                return ap[:, h * HW : (h + 1) * HW].rearrange(
                    "p (m c) -> p m c", c=1024
                )[:, :, g * HAG : (g + 1) * HAG]

            nunit = 0
            for g in range(2):
                for s in range(S):
                    for h in range(N // HW):
                        st = io.tile([P, QW], F32, tag="io")
                        stv = st[:].rearrange("p (m c) -> p m c", c=HAG)
                        nc.vector.scalar_tensor_tensor(
                            out=stv,
                            in0=c3(res_t[s], h, g),
                            scalar=isq_sb[:, s : s + 1],
                            in1=cb[g][h][:].rearrange(
                                "p (m c) -> p m c", c=HAG
                            ),
                            op0=MUL,
                            op1=MUL,
                        )
                        std = nc.sync if nunit % 2 == 0 else nc.scalar
                        std.dma_start(
                            c3(out[s * P : (s + 1) * P, :], h, g), stv
                        )
                        nunit += 1

    nc.compile()
    _CACHE["nc"] = nc
    return nc


def kernel(adjacency_matrix):
    A = np.ascontiguousarray(np.asarray(adjacency_matrix, dtype=np.float32))
    assert A.shape == (N, N)
    nc = build_nc()
    in_maps = [
        {"a_block": np.ascontiguousarray(A[k * R : (k + 1) * R])}
        for k in range(CORES)
    ]
    res = run_bass_kernel_spmd(nc, in_maps, list(range(CORES)))
    return np.concatenate(
        [res.results[k]["out_block"] for k in range(CORES)], axis=0
    )
